# revision 15
# baseline (speedup 1.0000x reference)
"""CRF-RNN mean-field iteration kernel for Trainium2 (8 NeuronCores).

Math (per batch b, NITERS=5):
    D_norm = W / W.sum(axis=1, keepdims)          # row-normalized affinity [n, n]
    qVals  = uniqs = seg.reshape(d, n)
    loop:  Q = softmax(qVals, axis=0)             # over class dim d=21
           seg_diff   = Q @ D_norm^T              # [d, n]
           seg_update = weights @ seg_diff
           qVals      = uniqs - seg_update

Sharding: batch b -> core pair (2b, 2b+1); each core owns half the output
positions (m rows of W). The contraction runs over all n, so W^T (contraction
index on partitions) is built on-device via PE transpose-matmuls against an
identity, quantized to fp8-e4m3, and kept resident in SBUF across all 5
iterations -- W is read from HBM exactly once, on both HWDGE rings (SP +
Activation) in parallel. The main matmuls run in fp8 DoubleRow mode (256-wide
contraction per pass). Row-normalization (1/rowsum, accumulated for free
during the fp32->fp8 cast on the Scalar engine) is applied per-partition to
the tiny seg_update output. Iteration 0 is emitted interleaved with the
(DMA-bound) transpose prepass so its matmuls hide under the HBM reads.

Per iteration the pair exchanges its half of softmax(Q) (64 KB fp8) via TWO
pairwise AllGathers: chunk A (own tiles 0-7) fires as soon as tail(1) has
produced it -- mid-iteration -- and chunk B (tiles 8-15) right after tail(3),
so the ~6us collective latency hides under the next iteration's own-half
matmuls. A dummy AllGather early in the prepass warms the CC stream (the
first collective otherwise pays ~12us of start delay). Sparse always-ready
filler matmuls (reading a resident junk tile) keep the PE_HAM activity window
from ever seeing a fully-idle 3.4us window, which would clock-gate the PE to
half rate. The instruction stream is identical on all cores (SPMD): all
own/partner asymmetry lives in host-side input permutations and a tiny
select-mask input.
"""

import os
import sys

for _p in ("/opt/trn_rl_repo",):
    if _p not in sys.path:
        sys.path.insert(0, _p)

import numpy as np

BS, D, RC = 4, 21, 64
N = RC * RC       # 4096 positions
NH = N // 2       # 2048 positions per core (own half)
NT = 32           # 128-wide position tiles (global)
NTO = 16          # own tiles
NT2 = 16          # 256-wide fp8 pair tiles (global)
NHALF = 8         # own/partner halves in NT2 tiles
SLABS = 16        # own-half m slabs of 128 rows
QPAD = 32         # class-dim padding for fp8 DoubleRow lhsT stride
NITERS = int(os.environ.get("CRF_NITERS", "5"))
NCORES = 8
RG = [[0, 1], [2, 3], [4, 5], [6, 7]]

PRE_FILL = int(os.environ.get("CRF_PRE_FILL", "10"))
IT_FILL = int(os.environ.get("CRF_IT_FILL", "8"))
EVAC_SCALAR = int(os.environ.get("CRF_EVAC_SCALAR", "2"))  # of 8 per slab

LAST_EXEC_NS = None
_CACHE = {}


def _install_ntff_hook():
    """Best-effort registration of the axon NTFF profile hook (image antenv
    lacks axon_hooks, so trn_boot could not register it)."""
    try:
        import types

        if "antenv.axon_hooks" in sys.modules:
            return
        holder = [None]
        m = types.ModuleType("antenv.axon_hooks")
        m.set_axon_ntff_profile_hook = lambda h: holder.__setitem__(0, h)
        m.get_axon_ntff_profile_hook = lambda: holder[0]
        sys.modules["antenv.axon_hooks"] = m
        import antenv

        antenv.axon_hooks = m
        from trn_agent_boot.trn_boot import _ntff_profile_via_ctypes

        m.set_axon_ntff_profile_hook(
            _ntff_profile_via_ctypes("/opt/axon/libaxon_pjrt.so")
        )
    except Exception:
        pass


def _build(niters):
    from concourse import bacc, bass, tile, mybir

    fp32, fp16 = mybir.dt.float32, mybir.dt.float16
    sdt = mybir.dt.float8e4
    qpad = QPAD
    AF = mybir.ActivationFunctionType
    ALU = mybir.AluOpType
    ntile = NT2
    half = NHALF
    perf = mybir.MatmulPerfMode.DoubleRow
    chunk = half * D  # fp8 elements per exchanged chunk per partition (no pad)

    nc = bacc.Bacc(None, target_bir_lowering=False)

    w_in = nc.dram_tensor("w", (NH, N), fp32, kind="ExternalInput")
    segt_in = nc.dram_tensor("segt", (128, NT, D), fp32, kind="ExternalInput")
    wt_in = nc.dram_tensor("wt", (D, D), fp32, kind="ExternalInput")
    sel_in = nc.dram_tensor("sel", (128, 2), fp32, kind="ExternalInput")
    id_in = nc.dram_tensor("ident", (128, 128), fp32, kind="ExternalInput")
    out_t = nc.dram_tensor("out", (128, NTO, D), fp32, kind="ExternalOutput")

    n_ex = max(0, niters - 1)
    ccA_ins = [
        nc.dram_tensor(f"ccA_in{k}", (128, chunk), sdt, kind="Internal")
        for k in range(n_ex)
    ]
    ccA_outs = [
        nc.dram_tensor(f"ccA_out{k}", (2, 128, chunk), sdt, kind="Internal")
        for k in range(n_ex)
    ]
    ccB_ins = [
        nc.dram_tensor(f"ccB_in{k}", (128, chunk), sdt, kind="Internal")
        for k in range(n_ex)
    ]
    ccB_outs = [
        nc.dram_tensor(f"ccB_out{k}", (2, 128, chunk), sdt, kind="Internal")
        for k in range(n_ex)
    ]
    n_warm = 3
    warm_ins = [
        nc.dram_tensor(f"warm_in{k}", (128, 32), sdt, kind="Internal")
        for k in range(n_warm)
    ]
    warm_outs = [
        nc.dram_tensor(f"warm_out{k}", (2, 128, 32), sdt, kind="Internal")
        for k in range(n_warm)
    ]

    with tile.TileContext(nc) as tc:
        with (
            tc.tile_pool(name="wt_res", bufs=1) as wt_res,
            tc.tile_pool(name="slab32", bufs=3) as slab32p,
            tc.tile_pool(name="slab8", bufs=3) as slab8p,
            tc.tile_pool(name="state", bufs=1) as state,
            tc.tile_pool(name="qt", bufs=2) as qtp,
            tc.tile_pool(name="work", bufs=2) as work,
            tc.tile_pool(name="ps_mm", bufs=1, space=bass.MemorySpace.PSUM) as ps_mm,
            tc.tile_pool(name="ps_misc", bufs=3, space=bass.MemorySpace.PSUM) as ps_misc,
            tc.tile_pool(name="ps_junk", bufs=1, space=bass.MemorySpace.PSUM) as ps_junk,
        ):
            # ---- small inputs (ACT HWDGE ring) ---------------------------
            id32 = state.tile([128, 128], fp32)
            nc.scalar.dma_start(id32[:], id_in[:])
            id_s = state.tile([128, 128], sdt)
            nc.vector.tensor_copy(id_s[:], id32[:])
            wt32 = state.tile([D, D], fp32)
            nc.scalar.dma_start(wt32[:], wt_in[:])
            wt16 = state.tile([D, D], fp16)
            nc.gpsimd.tensor_copy(wt16[:], wt32[:])
            segt = state.tile([128, NT, D], fp32)
            nc.scalar.dma_start(segt[:], segt_in[:])
            selt = state.tile([128, 2], fp32)
            nc.scalar.dma_start(selt[:], sel_in[:])
            zbias = state.tile([128, 1], fp32)
            nc.gpsimd.memset(zbias[:], 0.0)
            # mask for predicated partner select: nonzero where slot1=partner
            selmask = state.tile([128, chunk], mybir.dt.uint8)
            nc.gpsimd.tensor_scalar_mul(
                selmask[:],
                selt[:, 1:2].broadcast_to((128, chunk)),
                1.0,
            )
            # resident junk operand for HAM-keepalive fillers (never depends
            # on in-flight data, so queued fillers are always issueable)
            jrhs = state.tile([128, 768], sdt)
            nc.gpsimd.memset(jrhs[:], 0.0)

            # ---- CC-stream warmup: dummy pairwise AllGather --------------
            # The CC path goes cold after ~50us idle (the first collective
            # then pays ~15us extra), so fire keep-alives through the
            # prepass. All warmup DMAs ride the gpsimd software ring so they
            # never head-of-line-block the SP/ACT HWDGE rings.
            wa_s = state.tile([128, 32], sdt)
            nc.gpsimd.memset(wa_s[:], 0.0)
            nc.gpsimd.dma_start(warm_ins[0][:], wa_s[:])
            nc.gpsimd.collective_compute(
                "AllGather",
                ALU.bypass,
                replica_groups=RG,
                ins=[warm_ins[0][:].opt()],
                outs=[warm_outs[0][:].opt()],
            )

            def warm_keepalive(k, src_ap):
                # src_ap: a resident fp8 [128, 32] slice whose producer
                # naturally spaces this keep-alive within the prepass
                nc.gpsimd.dma_start(warm_ins[k][:], src_ap)
                nc.gpsimd.collective_compute(
                    "AllGather",
                    ALU.bypass,
                    replica_groups=RG,
                    ins=[warm_ins[k][:].opt()],
                    outs=[warm_outs[k][:].opt()],
                )

            def warm_consume():
                # consume the outputs (no DCE risk); fillers never read
                # jrhs[:, :128]
                for k in range(n_warm):
                    nc.gpsimd.dma_start(
                        jrhs[:, 32 * k : 32 * k + 32], warm_outs[k][0][:]
                    )

            # ---- initial Q = softmax(uniqs) over all 32 tiles ------------
            ex0 = state.tile([128, NT, D], fp32)
            nc.scalar.activation(ex0[:], segt[:], AF.Exp, bias=zbias[:])
            ssum0 = state.tile([128, NT], fp32)
            nc.vector.reduce_sum(ssum0[:], ex0[:], axis=mybir.AxisListType.X)
            srecip0 = state.tile([128, NT], fp32)
            nc.vector.reciprocal(srecip0[:], ssum0[:])
            qt_own = qtp.tile([128, NTO, qpad], sdt, tag="qt_own", name="qt_own0")
            qt_parA = qtp.tile([128, half, qpad], sdt, tag="qt_parA", name="qt_parA0")
            qt_parB = qtp.tile([128, half, qpad], sdt, tag="qt_parB", name="qt_parB0")
            nc.vector.tensor_tensor(
                qt_own[:, :, 0:D],
                ex0[:, 0:NTO, :],
                srecip0[:, 0:NTO, None].broadcast_to((128, NTO, D)),
                ALU.mult,
            )
            nc.vector.tensor_tensor(
                qt_parA[:, :, 0:D],
                ex0[:, NTO : NTO + half, :],
                srecip0[:, NTO : NTO + half, None].broadcast_to((128, half, D)),
                ALU.mult,
            )
            nc.vector.tensor_tensor(
                qt_parB[:, :, 0:D],
                ex0[:, NTO + half : NT, :],
                srecip0[:, NTO + half : NT, None].broadcast_to((128, half, D)),
                ALU.mult,
            )

            # ---- resident W^T (fp8, pair-interleaved for DoubleRow) ------
            # wt_mc[mc][p, t2, i, j] = W^T[256*t2 + 128*i + p, 512*mc + j]
            wt_mc = [
                wt_res.tile([128, NT2, 2, 512], sdt, tag=f"wtr{mc}", name=f"wt_mc{mc}")
                for mc in range(4)
            ]
            rs_colg = [
                state.tile([128, 4], fp32, tag=f"rscol{g}", name=f"rs_col{g}")
                for g in range(4)
            ]
            rs_recg = [
                state.tile([128, 4], fp32, tag=f"rsrec{g}", name=f"rs_rec{g}")
                for g in range(4)
            ]

            # scratch psum + filler matmuls: bound the PE idle windows so the
            # HAM clock gate never sees a fully-idle 3.4us window
            junk = ps_junk.tile([128, 512], fp32, name="junk")

            def fillers(n):
                for _ in range(n):
                    nc.tensor.matmul(
                        junk[:], jrhs[:, 128:256], jrhs[:, 256:768],
                        start=True, stop=True,
                    )

            def lhs_of(t, q_own, q_pA, q_pB):
                if t < half:
                    return q_own[:, 2 * t : 2 * t + 2, 0:D]
                if t < half + 4:
                    j2 = t - half
                    return q_pA[:, 2 * j2 : 2 * j2 + 2, 0:D]
                j2 = t - half - 4
                return q_pB[:, 2 * j2 : 2 * j2 + 2, 0:D]

            class IterEmitter:
                """Emits one mean-field iteration in dependency-friendly
                pieces so matmuls, evacuations, and the softmax tail
                pipeline across engines (and, for iteration 0, interleave
                with the prepass)."""

                def __init__(self, it, q_own, q_pA, q_pB, last):
                    self.it, self.last = it, last
                    self.q_own, self.q_pA, self.q_pB = q_own, q_pA, q_pB
                    self.pP = ps_mm.tile([D, NH], fp32, tag="pp", name=f"pp{it}")
                    self.ps16g = []
                    self.pUTg = []
                    self.qt_next = None
                    if not last:
                        self.qt_next = qtp.tile(
                            [128, NTO, qpad], sdt, tag="qt_own", name=f"qt_own{it+1}"
                        )

                def phase(self, mms):
                    for t, mc in mms:
                        nc.tensor.matmul(
                            self.pP[:, mc * 512 : (mc + 1) * 512],
                            lhs_of(t, self.q_own, self.q_pA, self.q_pB),
                            wt_mc[mc][:, t, :, :],
                            start=(t == 0),
                            stop=(t == ntile - 1),
                            perf_mode=perf,
                        )

                def evac(self, mc):
                    t16 = work.tile(
                        [D, 512], fp16, tag=f"ps16_{mc}", name=f"ps16_{self.it}_{mc}"
                    )
                    nc.vector.tensor_copy(t16[:], self.pP[:, mc * 512 : (mc + 1) * 512])
                    self.ps16g.append(t16)

                def ut(self, g):
                    pu = ps_misc.tile(
                        [128, 4 * D], fp32, tag="misc", name=f"pUT{self.it}_{g}"
                    )
                    for jj in range(4):
                        nc.tensor.matmul(
                            pu[:, jj * D : (jj + 1) * D],
                            self.ps16g[g][:, jj * 128 : (jj + 1) * 128],
                            wt16[:],
                            start=True,
                            stop=True,
                        )
                    self.pUTg.append(pu)

                def tail(self, g):
                    it, sl = self.it, slice(4 * g, 4 * g + 4)
                    upd = work.tile([128, 4, D], fp32, tag=f"upd{g}", name=f"upd{it}_{g}")
                    nc.vector.tensor_tensor(
                        upd[:],
                        self.pUTg[g][:].rearrange("p (a b) -> p a b", a=4),
                        rs_recg[g][:, :, None].broadcast_to((128, 4, D)),
                        ALU.mult,
                    )
                    qv = work.tile([128, 4, D], fp32, tag=f"qv{g}", name=f"qv{it}_{g}")
                    nc.vector.tensor_tensor(qv[:], segt[:, sl, :], upd[:], ALU.subtract)
                    if self.last:
                        nc.sync.dma_start(out_t[:, sl, :], qv[:])
                        return
                    exq = work.tile([128, 4, D], fp32, tag=f"exq{g}", name=f"exq{it}_{g}")
                    nc.scalar.activation(exq[:], qv[:], AF.Exp, bias=zbias[:])
                    ssum = work.tile([128, 4], fp32, tag=f"ssum{g}", name=f"ssum{it}_{g}")
                    nc.vector.reduce_sum(ssum[:], exq[:], axis=mybir.AxisListType.X)
                    srec = work.tile([128, 4], fp32, tag=f"srec{g}", name=f"srec{it}_{g}")
                    nc.vector.reciprocal(srec[:], ssum[:])
                    nc.vector.tensor_tensor(
                        self.qt_next[:, sl, 0:D],
                        exq[:],
                        srec[:, :, None].broadcast_to((128, 4, D)),
                        ALU.mult,
                    )

                def exchange_pre(self, which):
                    """Ship the first half of a chunk as soon as its tail is
                    done ('A': tail(0) -> tiles 0-3, 'B': tail(2) -> 8-11)."""
                    it = self.it
                    cin = (ccA_ins if which == "A" else ccB_ins)[it]
                    base = 0 if which == "A" else half
                    nc.sync.dma_start(
                        cin[:, 0 : 4 * D].rearrange("p (a b) -> p a b", a=4),
                        self.qt_next[:, base : base + 4, 0:D],
                    )

                def exchange(self, which):
                    """Fire one half-exchange. 'A' needs tails 0-1 done,
                    'B' needs tails 2-3 done."""
                    it = self.it
                    cin = (ccA_ins if which == "A" else ccB_ins)[it]
                    cout = (ccA_outs if which == "A" else ccB_outs)[it]
                    base = 0 if which == "A" else half
                    nc.sync.dma_start(
                        cin[:, 4 * D : chunk].rearrange("p (a b) -> p a b", a=4),
                        self.qt_next[:, base + 4 : base + half, 0:D],
                    )
                    nc.gpsimd.collective_compute(
                        "AllGather",
                        ALU.bypass,
                        replica_groups=RG,
                        ins=[cin[:].opt()],
                        outs=[cout[:].opt()],
                    )
                    tag = f"qt_par{which}"
                    qt_par_next = qtp.tile(
                        [128, half, qpad], sdt, tag=tag, name=f"{tag}{it+1}"
                    )
                    g1 = work.tile(
                        [128, chunk], sdt, tag=f"g1{which}", name=f"g1{which}_{it}"
                    )
                    nc.sync.dma_start(
                        qt_par_next[:, :, 0:D],
                        cout[0][:].rearrange("p (a b) -> p a b", a=half),
                    )
                    nc.scalar.dma_start(g1[:], cout[1][:])
                    nc.vector.copy_predicated(
                        qt_par_next[:, :, 0:D],
                        selmask[:].rearrange("p (a b) -> p a b", a=half),
                        g1[:].rearrange("p (a b) -> p a b", a=half),
                    )
                    return qt_par_next

            # ---- prepass (slabs, transpose, rowsum) + iteration 0 --------
            em = IterEmitter(0, qt_own, qt_parA, qt_parB, last=(niters == 1))
            nxA = nxB = None
            for ms in range(SLABS):
                if ms == 10:
                    warm_keepalive(1, wt_mc[2][:, 0, 0, 0:32])
                if ms == 14:
                    warm_keepalive(2, wt_mc[3][:, 0, 0, 0:32])
                w32 = slab32p.tile([128, N], fp32, tag="w32", name=f"w32_{ms}")
                nc.sync.dma_start(w32[:], w_in[ms * 128 : (ms + 1) * 128, :])
                w8 = slab8p.tile([128, N], sdt, tag="w8", name=f"w8_{ms}")
                nc.scalar.activation(
                    w8[:], w32[:], AF.Copy,
                    accum_out=rs_colg[ms // 4][:, ms % 4 : ms % 4 + 1],
                )
                mc, col = ms // 4, (ms % 4) * 128
                for g in range(8):
                    ptp = ps_misc.tile([128, 512], fp32, tag="misc", name=f"ptp{ms}_{g}")
                    for k2 in range(4):
                        nt = 4 * g + k2
                        nc.tensor.matmul(
                            ptp[:, k2 * 128 : (k2 + 1) * 128],
                            w8[:, nt * 128 : (nt + 1) * 128],
                            id_s[:],
                            start=True,
                            stop=True,
                        )
                    dst = wt_mc[mc][:, 2 * g : 2 * g + 2, :, col : col + 128]
                    src = ptp[:].rearrange("p (a b c) -> p a b c", a=2, b=2)
                    if g < EVAC_SCALAR:
                        nc.scalar.activation(dst, src, AF.Copy)
                    else:
                        nc.vector.tensor_copy(dst, src)
                fillers(PRE_FILL)
                if ms % 4 == 3:
                    g = ms // 4
                    nc.vector.reciprocal(rs_recg[g][:], rs_colg[g][:])
                    em.phase([(t, g) for t in range(ntile)])
                    em.evac(g)
                    if g >= 1:
                        em.ut(g - 1)
                        em.tail(g - 1)
                        if niters > 1:
                            if g == 1:
                                em.exchange_pre("A")
                            elif g == 2:
                                nxA = em.exchange("A")
                            elif g == 3:
                                em.exchange_pre("B")
            warm_consume()
            em.ut(3)
            em.tail(3)
            if niters > 1:
                nxB = em.exchange("B")
                qt_own, qt_parA, qt_parB = em.qt_next, nxA, nxB

            # ---- iterations 1..niters-1 ---------------------------------
            for it in range(1, niters):
                em = IterEmitter(it, qt_own, qt_parA, qt_parB, last=(it == niters - 1))
                em.phase([(t, mc) for t in range(half) for mc in range(4)])
                fillers(IT_FILL)
                em.phase([(t, mc) for t in range(half, half + 4) for mc in range(4)])
                fillers(IT_FILL)
                nxA = nxB = None
                for mc in range(4):
                    em.phase([(t, mc) for t in range(half + 4, ntile)])
                    em.evac(mc)
                    if mc >= 1:
                        em.ut(mc - 1)
                        em.tail(mc - 1)
                        if not em.last:
                            if mc == 1:
                                em.exchange_pre("A")
                            elif mc == 2:
                                nxA = em.exchange("A")
                            elif mc == 3:
                                em.exchange_pre("B")
                em.ut(3)
                em.tail(3)
                if not em.last:
                    nxB = em.exchange("B")
                    qt_own, qt_parA, qt_parB = em.qt_next, nxA, nxB

    nc.compile()
    return nc


def _get_nc(niters):
    if niters not in _CACHE:
        _CACHE[niters] = _build(niters)
    return _CACHE[niters]


def kernel(seg, W, weights):
    global LAST_EXEC_NS
    assert seg.shape == (BS, D, RC, RC) and W.shape == (BS, N, N)
    trace = bool(os.environ.get("BASS_TRACE"))
    if trace:
        _install_ntff_hook()

    from concourse.bass_utils import run_bass_kernel_spmd

    nc = _get_nc(NITERS)

    seg32 = np.ascontiguousarray(seg, dtype=np.float32)
    W32 = np.ascontiguousarray(W, dtype=np.float32)
    wt_np = np.ascontiguousarray(weights.T, dtype=np.float32)
    id_np = np.eye(128, dtype=np.float32)

    in_maps = []
    for c in range(NCORES):
        b, h = c // 2, c % 2
        own = slice(NH * h, NH * h + NH)
        par = slice(NH * (1 - h), NH * (1 - h) + NH)
        Wb = W32[b]
        w_np = np.ascontiguousarray(
            np.concatenate([Wb[own, own], Wb[own, par]], axis=1)
        )
        st = seg32[b].reshape(D, N).T  # [n, d]
        st_perm = np.concatenate([st[own], st[par]], axis=0)
        segt_np = np.ascontiguousarray(
            st_perm.reshape(NT, 128, D).transpose(1, 0, 2)
        )
        sel_np = np.zeros((128, 2), np.float32)
        sel_np[:, 0] = float(h)       # gather slot (1-h) = partner
        sel_np[:, 1] = float(1 - h)
        in_maps.append(
            {"w": w_np, "segt": segt_np, "wt": wt_np, "sel": sel_np, "ident": id_np}
        )

    res = run_bass_kernel_spmd(
        nc, in_maps, core_ids=list(range(NCORES)), trace=trace
    )
    LAST_EXEC_NS = res.exec_time_ns

    out = np.empty((BS, D, N), np.float32)
    for c in range(NCORES):
        b, h = c // 2, c % 2
        qv = res.results[c]["out"]  # [128, NTO, D]
        block = qv.transpose(2, 1, 0).reshape(D, NH)
        out[b][:, NH * h : NH * h + NH] = block
    return out.reshape(BS, D, RC, RC)


if __name__ == "__main__":
    rng = np.random.default_rng(0)
    seg = rng.standard_normal((BS, D, RC, RC)).astype(np.float32)
    W = rng.random((BS, N, N), dtype=np.float32)
    weights = rng.standard_normal((D, D)).astype(np.float32)
    out = kernel(seg=seg, W=W, weights=weights)
    print("out", out.shape, out.dtype, float(np.abs(out).mean()))


# revision 25
# speedup vs baseline: 1.0328x; 1.0328x over previous
"""CRF-RNN mean-field iteration kernel for Trainium2 (8 NeuronCores).

Math (per batch b, NITERS=5):
    D_norm = W / W.sum(axis=1, keepdims)          # row-normalized affinity [n, n]
    qVals  = uniqs = seg.reshape(d, n)
    loop:  Q = softmax(qVals, axis=0)             # over class dim d=21
           seg_diff   = Q @ D_norm^T              # [d, n]
           seg_update = weights @ seg_diff
           qVals      = uniqs - seg_update

Sharding: batch b -> core pair (2b, 2b+1); each core owns half the output
positions (m rows of W). The contraction runs over all n, so W^T (contraction
index on partitions) is built on-device via PE transpose-matmuls against an
identity, quantized to fp8-e4m3, and kept resident in SBUF across all 5
iterations -- W is read from HBM exactly once, on both HWDGE rings (SP +
Activation) in parallel. The main matmuls run in fp8 DoubleRow mode (256-wide
contraction per pass). Row-normalization (1/rowsum, accumulated for free
during the fp32->fp8 cast on the Scalar engine) is applied per-partition to
the tiny seg_update output. Iteration 0 is emitted interleaved with the
(DMA-bound) transpose prepass so its matmuls hide under the HBM reads.

Per iteration the pair exchanges its half of softmax(Q) (64 KB fp8) via TWO
pairwise AllGathers: chunk A (own tiles 0-7) fires as soon as tail(1) has
produced it -- mid-iteration -- and chunk B (tiles 8-15) right after tail(3),
so the ~6us collective latency hides under the next iteration's own-half
matmuls. A dummy AllGather early in the prepass warms the CC stream (the
first collective otherwise pays ~12us of start delay). Sparse always-ready
filler matmuls (reading a resident junk tile) keep the PE_HAM activity window
from ever seeing a fully-idle 3.4us window, which would clock-gate the PE to
half rate. The instruction stream is identical on all cores (SPMD): all
own/partner asymmetry lives in host-side input permutations and a tiny
select-mask input.
"""

import os
import sys

for _p in ("/opt/trn_rl_repo",):
    if _p not in sys.path:
        sys.path.insert(0, _p)

import numpy as np

BS, D, RC = 4, 21, 64
N = RC * RC       # 4096 positions
NH = N // 2       # 2048 positions per core (own half)
NT = 32           # 128-wide position tiles (global)
NTO = 16          # own tiles
NT2 = 16          # 256-wide fp8 pair tiles (global)
NHALF = 8         # own/partner halves in NT2 tiles
SLABS = 16        # own-half m slabs of 128 rows
QPAD = 32         # class-dim padding for fp8 DoubleRow lhsT stride
NITERS = int(os.environ.get("CRF_NITERS", "5"))
NCORES = 8
RG = [[0, 1], [2, 3], [4, 5], [6, 7]]

PRE_FILL = int(os.environ.get("CRF_PRE_FILL", "10"))
IT_FILL = int(os.environ.get("CRF_IT_FILL", "14"))
EVAC_SCALAR = int(os.environ.get("CRF_EVAC_SCALAR", "2"))  # of 8 per slab

LAST_EXEC_NS = None
_CACHE = {}


def _install_ntff_hook():
    """Best-effort registration of the axon NTFF profile hook (image antenv
    lacks axon_hooks, so trn_boot could not register it)."""
    try:
        import types

        if "antenv.axon_hooks" in sys.modules:
            return
        holder = [None]
        m = types.ModuleType("antenv.axon_hooks")
        m.set_axon_ntff_profile_hook = lambda h: holder.__setitem__(0, h)
        m.get_axon_ntff_profile_hook = lambda: holder[0]
        sys.modules["antenv.axon_hooks"] = m
        import antenv

        antenv.axon_hooks = m
        from trn_agent_boot.trn_boot import _ntff_profile_via_ctypes

        m.set_axon_ntff_profile_hook(
            _ntff_profile_via_ctypes("/opt/axon/libaxon_pjrt.so")
        )
    except Exception:
        pass


def _build(niters):
    from concourse import bacc, bass, tile, mybir

    fp32, fp16 = mybir.dt.float32, mybir.dt.float16
    sdt = mybir.dt.float8e4
    qpad = QPAD
    AF = mybir.ActivationFunctionType
    ALU = mybir.AluOpType
    ntile = NT2
    half = NHALF
    perf = mybir.MatmulPerfMode.DoubleRow
    chunk = NTO * D  # fp8 elements exchanged per partition (no pad)

    nc = bacc.Bacc(None, target_bir_lowering=False)

    w_in = nc.dram_tensor("w", (NH, N), fp32, kind="ExternalInput")
    segt_in = nc.dram_tensor("segt", (128, NT, D), fp32, kind="ExternalInput")
    wt_in = nc.dram_tensor("wt", (D, D), fp32, kind="ExternalInput")
    sel_in = nc.dram_tensor("sel", (128, 2), fp32, kind="ExternalInput")
    id_in = nc.dram_tensor("ident", (128, 128), fp32, kind="ExternalInput")
    out_t = nc.dram_tensor("out", (128, NTO, D), fp32, kind="ExternalOutput")

    n_ex = max(0, niters - 1)
    cc_ins = [
        nc.dram_tensor(f"cc_in{k}", (128, chunk), sdt, kind="Internal")
        for k in range(n_ex)
    ]
    cc_outs = [
        nc.dram_tensor(f"cc_out{k}", (2, 128, chunk), sdt, kind="Internal")
        for k in range(n_ex)
    ]
    n_warm = 3
    warm_ins = [
        nc.dram_tensor(f"warm_in{k}", (128, 32), sdt, kind="Internal")
        for k in range(n_warm)
    ]
    warm_outs = [
        nc.dram_tensor(f"warm_out{k}", (2, 128, 32), sdt, kind="Internal")
        for k in range(n_warm)
    ]

    with tile.TileContext(nc) as tc:
        with (
            tc.tile_pool(name="wt_res", bufs=1) as wt_res,
            tc.tile_pool(name="slab32", bufs=3) as slab32p,
            tc.tile_pool(name="slab8", bufs=3) as slab8p,
            tc.tile_pool(name="state", bufs=1) as state,
            tc.tile_pool(name="qt", bufs=2) as qtp,
            tc.tile_pool(name="work", bufs=2) as work,
            tc.tile_pool(name="ps_mm", bufs=1, space=bass.MemorySpace.PSUM) as ps_mm,
            tc.tile_pool(name="ps_misc", bufs=3, space=bass.MemorySpace.PSUM) as ps_misc,
            tc.tile_pool(name="ps_junk", bufs=1, space=bass.MemorySpace.PSUM) as ps_junk,
        ):
            # ---- small inputs (ACT HWDGE ring) ---------------------------
            id32 = state.tile([128, 128], fp32)
            nc.scalar.dma_start(id32[:], id_in[:])
            id_s = state.tile([128, 128], sdt)
            nc.vector.tensor_copy(id_s[:], id32[:])
            wt32 = state.tile([D, D], fp32)
            nc.scalar.dma_start(wt32[:], wt_in[:])
            wt16 = state.tile([D, D], fp16)
            nc.gpsimd.tensor_copy(wt16[:], wt32[:])
            segt = state.tile([128, NT, D], fp32)
            nc.scalar.dma_start(segt[:], segt_in[:])
            selt = state.tile([128, 2], fp32)
            nc.scalar.dma_start(selt[:], sel_in[:])
            zbias = state.tile([128, 1], fp32)
            nc.gpsimd.memset(zbias[:], 0.0)
            # mask for predicated partner select: nonzero where slot1=partner
            selmask = state.tile([128, chunk], mybir.dt.uint8)
            nc.gpsimd.tensor_scalar_mul(
                selmask[:],
                selt[:, 1:2].broadcast_to((128, chunk)),
                1.0,
            )
            # resident junk operand for HAM-keepalive fillers (never depends
            # on in-flight data, so queued fillers are always issueable)
            jrhs = state.tile([128, 768], sdt)
            nc.gpsimd.memset(jrhs[:], 0.0)

            # ---- CC-stream warmup: dummy pairwise AllGather --------------
            # The CC path goes cold after ~50us idle (the first collective
            # then pays ~15us extra), so fire keep-alives through the
            # prepass. All warmup DMAs ride the gpsimd software ring so they
            # never head-of-line-block the SP/ACT HWDGE rings.
            wa_s = state.tile([128, 32], sdt)
            nc.gpsimd.memset(wa_s[:], 0.0)
            nc.gpsimd.dma_start(warm_ins[0][:], wa_s[:])
            nc.gpsimd.collective_compute(
                "AllGather",
                ALU.bypass,
                replica_groups=RG,
                ins=[warm_ins[0][:].opt()],
                outs=[warm_outs[0][:].opt()],
            )

            def warm_keepalive(k, src_ap):
                # src_ap: a resident fp8 [128, 32] slice whose producer
                # naturally spaces this keep-alive within the prepass
                nc.gpsimd.dma_start(warm_ins[k][:], src_ap)
                nc.gpsimd.collective_compute(
                    "AllGather",
                    ALU.bypass,
                    replica_groups=RG,
                    ins=[warm_ins[k][:].opt()],
                    outs=[warm_outs[k][:].opt()],
                )

            def warm_consume():
                # consume the outputs (no DCE risk); fillers never read
                # jrhs[:, :128]
                for k in range(n_warm):
                    nc.gpsimd.dma_start(
                        jrhs[:, 32 * k : 32 * k + 32], warm_outs[k][0][:]
                    )

            # ---- initial Q = softmax(uniqs) over all 32 tiles ------------
            ex0 = state.tile([128, NT, D], fp32)
            nc.scalar.activation(ex0[:], segt[:], AF.Exp, bias=zbias[:])
            ssum0 = state.tile([128, NT], fp32)
            nc.vector.reduce_sum(ssum0[:], ex0[:], axis=mybir.AxisListType.X)
            srecip0 = state.tile([128, NT], fp32)
            nc.vector.reciprocal(srecip0[:], ssum0[:])
            qt_own = qtp.tile([128, NTO, qpad], sdt, tag="qt_own", name="qt_own0")
            qt_par = qtp.tile([128, NTO, qpad], sdt, tag="qt_par", name="qt_par0")
            nc.vector.tensor_tensor(
                qt_own[:, :, 0:D],
                ex0[:, 0:NTO, :],
                srecip0[:, 0:NTO, None].broadcast_to((128, NTO, D)),
                ALU.mult,
            )
            nc.vector.tensor_tensor(
                qt_par[:, :, 0:D],
                ex0[:, NTO:NT, :],
                srecip0[:, NTO:NT, None].broadcast_to((128, NTO, D)),
                ALU.mult,
            )

            # ---- resident W^T (fp8, pair-interleaved for DoubleRow) ------
            # wt_mc[mc][p, t2, i, j] = W^T[256*t2 + 128*i + p, 512*mc + j]
            wt_mc = [
                wt_res.tile([128, NT2, 2, 512], sdt, tag=f"wtr{mc}", name=f"wt_mc{mc}")
                for mc in range(4)
            ]
            rs_colg = [
                state.tile([128, 4], fp32, tag=f"rscol{g}", name=f"rs_col{g}")
                for g in range(4)
            ]
            rs_recg = [
                state.tile([128, 4], fp32, tag=f"rsrec{g}", name=f"rs_rec{g}")
                for g in range(4)
            ]

            # scratch psum + filler matmuls: bound the PE idle windows so the
            # HAM clock gate never sees a fully-idle 3.4us window
            junk = ps_junk.tile([128, 512], fp32, name="junk")

            def fillers(n):
                for _ in range(n):
                    nc.tensor.matmul(
                        junk[:], jrhs[:, 128:256], jrhs[:, 256:768],
                        start=True, stop=True,
                    )

            def lhs_of(t, q_own, q_par):
                src = q_own if t < half else q_par
                j2 = t % half
                return src[:, 2 * j2 : 2 * j2 + 2, 0:D]

            class IterEmitter:
                """Emits one mean-field iteration in dependency-friendly
                pieces so matmuls, evacuations, and the softmax tail
                pipeline across engines (and, for iteration 0, interleave
                with the prepass)."""

                def __init__(self, it, q_own, q_par, last):
                    self.it, self.last = it, last
                    self.q_own, self.q_par = q_own, q_par
                    self.pP = ps_mm.tile([D, NH], fp32, tag="pp", name=f"pp{it}")
                    self.ps16g = []
                    self.pUTg = []
                    self.qt_next = None
                    if not last:
                        self.qt_next = qtp.tile(
                            [128, NTO, qpad], sdt, tag="qt_own", name=f"qt_own{it+1}"
                        )

                def phase(self, mms):
                    for t, mc in mms:
                        nc.tensor.matmul(
                            self.pP[:, mc * 512 : (mc + 1) * 512],
                            lhs_of(t, self.q_own, self.q_par),
                            wt_mc[mc][:, t, :, :],
                            start=(t == 0),
                            stop=(t == ntile - 1),
                            perf_mode=perf,
                        )

                def evac(self, mc):
                    t16 = work.tile(
                        [D, 512], fp16, tag=f"ps16_{mc}", name=f"ps16_{self.it}_{mc}"
                    )
                    nc.vector.tensor_copy(t16[:], self.pP[:, mc * 512 : (mc + 1) * 512])
                    self.ps16g.append(t16)

                def ut(self, g):
                    pu = ps_misc.tile(
                        [128, 4 * D], fp32, tag="misc", name=f"pUT{self.it}_{g}"
                    )
                    for jj in range(4):
                        nc.tensor.matmul(
                            pu[:, jj * D : (jj + 1) * D],
                            self.ps16g[g][:, jj * 128 : (jj + 1) * 128],
                            wt16[:],
                            start=True,
                            stop=True,
                        )
                    self.pUTg.append(pu)

                def tail(self, g):
                    it, sl = self.it, slice(4 * g, 4 * g + 4)
                    upd = work.tile([128, 4, D], fp32, tag=f"upd{g}", name=f"upd{it}_{g}")
                    nc.vector.tensor_tensor(
                        upd[:],
                        self.pUTg[g][:].rearrange("p (a b) -> p a b", a=4),
                        rs_recg[g][:, :, None].broadcast_to((128, 4, D)),
                        ALU.mult,
                    )
                    qv = work.tile([128, 4, D], fp32, tag=f"qv{g}", name=f"qv{it}_{g}")
                    nc.vector.tensor_tensor(qv[:], segt[:, sl, :], upd[:], ALU.subtract)
                    if self.last:
                        nc.sync.dma_start(out_t[:, sl, :], qv[:])
                        return
                    exq = work.tile([128, 4, D], fp32, tag=f"exq{g}", name=f"exq{it}_{g}")
                    nc.scalar.activation(exq[:], qv[:], AF.Exp, bias=zbias[:])
                    ssum = work.tile([128, 4], fp32, tag=f"ssum{g}", name=f"ssum{it}_{g}")
                    nc.vector.reduce_sum(ssum[:], exq[:], axis=mybir.AxisListType.X)
                    srec = work.tile([128, 4], fp32, tag=f"srec{g}", name=f"srec{it}_{g}")
                    nc.vector.reciprocal(srec[:], ssum[:])
                    nc.vector.tensor_tensor(
                        self.qt_next[:, sl, 0:D],
                        exq[:],
                        srec[:, :, None].broadcast_to((128, 4, D)),
                        ALU.mult,
                    )

                def ship(self, g):
                    """Ship tail(g)'s 4 tiles of qt_next into this
                    iteration's collective input buffer (staged, so the
                    post-tail(3) trigger only waits on a 10KB DMA)."""
                    cin = cc_ins[self.it]
                    nc.sync.dma_start(
                        cin[:, 4 * g * D : (4 * g + 4) * D].rearrange(
                            "p (a b) -> p a b", a=4
                        ),
                        self.qt_next[:, 4 * g : 4 * g + 4, 0:D],
                    )

                def exchange(self):
                    """Fire the exchange (all four ship()s done)."""
                    it = self.it
                    cin, cout = cc_ins[it], cc_outs[it]
                    nc.gpsimd.collective_compute(
                        "AllGather",
                        ALU.bypass,
                        replica_groups=RG,
                        ins=[cin[:].opt()],
                        outs=[cout[:].opt()],
                    )
                    qt_par_next = qtp.tile(
                        [128, NTO, qpad], sdt, tag="qt_par", name=f"qt_par{it+1}"
                    )
                    g1 = work.tile([128, chunk], sdt, tag="g1", name=f"g1_{it}")
                    hc = chunk // 2
                    # halves ride separate rings and merge separately so the
                    # first partner tiles unlock before the whole 42KB lands
                    for hh in range(2):
                        sl = slice(hh * half, hh * half + half)
                        cs = slice(hh * hc, hh * hc + hc)
                        nc.sync.dma_start(
                            qt_par_next[:, sl, 0:D],
                            cout[0][:, cs].rearrange("p (a b) -> p a b", a=half),
                        )
                        nc.scalar.dma_start(g1[:, cs], cout[1][:, cs])
                        nc.vector.copy_predicated(
                            qt_par_next[:, sl, 0:D],
                            selmask[:, cs].rearrange("p (a b) -> p a b", a=half),
                            g1[:, cs].rearrange("p (a b) -> p a b", a=half),
                        )
                    return qt_par_next

            # ---- prepass (slabs, transpose, rowsum) + iteration 0 --------
            em = IterEmitter(0, qt_own, qt_par, last=(niters == 1))
            for ms in range(SLABS):
                if ms == 10:
                    warm_keepalive(1, wt_mc[2][:, 0, 0, 0:32])
                if ms == 14:
                    warm_keepalive(2, wt_mc[3][:, 0, 0, 0:32])
                w32 = slab32p.tile([128, N], fp32, tag="w32", name=f"w32_{ms}")
                nc.sync.dma_start(w32[:], w_in[ms * 128 : (ms + 1) * 128, :])
                w8 = slab8p.tile([128, N], sdt, tag="w8", name=f"w8_{ms}")
                nc.scalar.activation(
                    w8[:], w32[:], AF.Copy,
                    accum_out=rs_colg[ms // 4][:, ms % 4 : ms % 4 + 1],
                )
                mc, col = ms // 4, (ms % 4) * 128
                for g in range(8):
                    ptp = ps_misc.tile([128, 512], fp32, tag="misc", name=f"ptp{ms}_{g}")
                    for k2 in range(4):
                        nt = 4 * g + k2
                        nc.tensor.matmul(
                            ptp[:, k2 * 128 : (k2 + 1) * 128],
                            w8[:, nt * 128 : (nt + 1) * 128],
                            id_s[:],
                            start=True,
                            stop=True,
                        )
                    dst = wt_mc[mc][:, 2 * g : 2 * g + 2, :, col : col + 128]
                    src = ptp[:].rearrange("p (a b c) -> p a b c", a=2, b=2)
                    if g < EVAC_SCALAR:
                        nc.scalar.activation(dst, src, AF.Copy)
                    else:
                        nc.vector.tensor_copy(dst, src)
                fillers(PRE_FILL)
                if ms % 4 == 3:
                    g = ms // 4
                    nc.vector.reciprocal(rs_recg[g][:], rs_colg[g][:])
                    em.phase([(t, g) for t in range(ntile)])
                    em.evac(g)
                    if g >= 1:
                        em.ut(g - 1)
                        em.tail(g - 1)
                        if niters > 1:
                            em.ship(g - 1)
            warm_consume()
            em.ut(3)
            em.tail(3)
            if niters > 1:
                em.ship(3)
                qt_par = em.exchange()
                qt_own = em.qt_next

            # ---- iterations 1..niters-1 ---------------------------------
            for it in range(1, niters):
                em = IterEmitter(it, qt_own, qt_par, last=(it == niters - 1))
                em.phase([(t, mc) for t in range(half) for mc in range(4)])
                fillers(IT_FILL)
                for mc in range(4):
                    em.phase([(t, mc) for t in range(half, ntile)])
                    em.evac(mc)
                    if mc >= 1:
                        em.ut(mc - 1)
                        em.tail(mc - 1)
                        if not em.last:
                            em.ship(mc - 1)
                em.ut(3)
                em.tail(3)
                if not em.last:
                    em.ship(3)
                    qt_par = em.exchange()
                    qt_own = em.qt_next

    nc.compile()
    return nc


def _get_nc(niters):
    if niters not in _CACHE:
        _CACHE[niters] = _build(niters)
    return _CACHE[niters]


def kernel(seg, W, weights):
    global LAST_EXEC_NS
    assert seg.shape == (BS, D, RC, RC) and W.shape == (BS, N, N)
    trace = bool(os.environ.get("BASS_TRACE"))
    if trace:
        _install_ntff_hook()

    from concourse.bass_utils import run_bass_kernel_spmd

    nc = _get_nc(NITERS)

    seg32 = np.ascontiguousarray(seg, dtype=np.float32)
    W32 = np.ascontiguousarray(W, dtype=np.float32)
    wt_np = np.ascontiguousarray(weights.T, dtype=np.float32)
    id_np = np.eye(128, dtype=np.float32)

    in_maps = []
    for c in range(NCORES):
        b, h = c // 2, c % 2
        own = slice(NH * h, NH * h + NH)
        par = slice(NH * (1 - h), NH * (1 - h) + NH)
        Wb = W32[b]
        w_np = np.ascontiguousarray(
            np.concatenate([Wb[own, own], Wb[own, par]], axis=1)
        )
        st = seg32[b].reshape(D, N).T  # [n, d]
        st_perm = np.concatenate([st[own], st[par]], axis=0)
        segt_np = np.ascontiguousarray(
            st_perm.reshape(NT, 128, D).transpose(1, 0, 2)
        )
        sel_np = np.zeros((128, 2), np.float32)
        sel_np[:, 0] = float(h)       # gather slot (1-h) = partner
        sel_np[:, 1] = float(1 - h)
        in_maps.append(
            {"w": w_np, "segt": segt_np, "wt": wt_np, "sel": sel_np, "ident": id_np}
        )

    res = run_bass_kernel_spmd(
        nc, in_maps, core_ids=list(range(NCORES)), trace=trace
    )
    LAST_EXEC_NS = res.exec_time_ns

    out = np.empty((BS, D, N), np.float32)
    for c in range(NCORES):
        b, h = c // 2, c % 2
        qv = res.results[c]["out"]  # [128, NTO, D]
        block = qv.transpose(2, 1, 0).reshape(D, NH)
        out[b][:, NH * h : NH * h + NH] = block
    return out.reshape(BS, D, RC, RC)


if __name__ == "__main__":
    rng = np.random.default_rng(0)
    seg = rng.standard_normal((BS, D, RC, RC)).astype(np.float32)
    W = rng.random((BS, N, N), dtype=np.float32)
    weights = rng.standard_normal((D, D)).astype(np.float32)
    out = kernel(seg=seg, W=W, weights=weights)
    print("out", out.shape, out.dtype, float(np.abs(out).mean()))
